# revision 14
# baseline (speedup 1.0000x reference)
"""Trainium2 Bass kernel for CheemsNonWoAttention (GQA attention, no output proj).

Sharding: 16 q-heads across 8 cores (2 q-heads + their shared kv-head per
core), SPMD with no collectives.  Each core computes its slice of the output
hidden dim; the host concatenates.

Math notes:
  - The reference's logn scale is max(log(65..80)/log(256), 1) == 1.0 -> no-op.
  - 1/sqrt(HD) score scale is folded into Wq on the host.
  - Softmax runs without max-subtraction (scores ~ N(0,1) + additive mask;
    exp underflows to 0 for very negative masks, which is exactly right).
    Scores are computed transposed, sT[k, q], so exp(sT) feeds attn@V
    directly as the moving operand (V chunks stationary), denominators come
    from a ones-vector matmul, and only the final [d, q] -> [q, d] flip
    needs PE transposes.
  - Matmuls run in float32r (TF32-like, ~1.5e-4 rms rel err).
  - The host inspects the mask and dispatches one of three compiled
    variants: "causal" (skips fully-masked k-chunks and the mask add on
    fully-unmasked ones), "zeros" (no mask work at all), "general"
    (arbitrary additive mask).
"""

import sys

if "/opt/trn_rl_repo" not in sys.path:
    sys.path.insert(0, "/opt/trn_rl_repo")

import math
import numpy as np

B, S, HID = 2, 2048, 2048
NH, NKV, HD = 16, 4, 128
NCORES = 8
HPC = NH // NCORES          # q heads per core
FPC = HPC * HD              # output features per core
KVW = HD                    # kv head width per core
P = 128
NCH = HID // P              # hid chunks (contraction tiles)
TT = 512                    # token tile, phase 1
QT = 512                    # q tile, phase 2
NKC = S // P                # k chunks

_CACHE = {}


def _patch_ldw_opt():
    # walrus's LDWEIGHTS dedup/overlap pass is off by default in the driver
    # args; it is worth ~13% end-to-end here (weight loads dominate fp32r
    # matmul issue otherwise).  Results verified identical with it on.
    import concourse.bass_utils as bu

    if getattr(bu, "_ldw_opt_patched", False):
        return
    orig = bu.run_command

    def patched(argv, **kw):
        argv = ["--enable-ldw-opt=true" if a == "--enable-ldw-opt=false" else a
                for a in argv]
        return orig(argv, **kw)

    bu.run_command = patched
    bu._ldw_opt_patched = True


def _build_nc(variant):
    _patch_ldw_opt()
    import concourse.bacc as bacc
    from concourse import mybir
    from concourse.tile import TileContext

    f32 = mybir.dt.float32
    f32r = mybir.dt.float32r
    bf16 = mybir.dt.bfloat16
    Exp = mybir.ActivationFunctionType.Exp

    nc = bacc.Bacc("TRN2", target_bir_lowering=False, debug=False, num_devices=NCORES)
    xT = nc.dram_tensor("xT", [B, HID, S], f32r, kind="ExternalInput").ap()
    wq = nc.dram_tensor("wq", [HID, FPC], f32r, kind="ExternalInput").ap()
    wk = nc.dram_tensor("wk", [HID, KVW], f32r, kind="ExternalInput").ap()
    wv = nc.dram_tensor("wv", [HID, KVW], f32r, kind="ExternalInput").ap()
    maskT = nc.dram_tensor("maskT", [B, S, S], bf16, kind="ExternalInput").ap()
    ident_d = nc.dram_tensor("ident", [P, P], f32r, kind="ExternalInput").ap()
    ident32_d = nc.dram_tensor("ident32", [P, P], f32, kind="ExternalInput").ap()
    ones_d = nc.dram_tensor("ones", [P, 1], f32r, kind="ExternalInput").ap()
    out = nc.dram_tensor("out", [B, S, FPC], f32, kind="ExternalOutput").ap()

    def active_kchunks(q0):
        if variant == "causal":
            return list(range(q0 // P + QT // P))
        return list(range(NKC))

    def masked_kchunks(q0):
        if variant == "causal":
            return set(range(q0 // P, q0 // P + QT // P))
        if variant == "zeros":
            return set()
        return set(range(NKC))

    with TileContext(nc) as tc:
        with tc.tile_pool(name="persist", bufs=1) as persist:
            wq_sb = persist.tile([P, NCH, FPC], f32r, tag="wq")
            wk_sb = persist.tile([P, NCH, KVW], f32r, tag="wk")
            wv_sb = persist.tile([P, NCH, KVW], f32r, tag="wv")
            ident = persist.tile([P, P], f32r, tag="ident")
            ident32 = persist.tile([P, P], f32, tag="ident32")
            ones_sb = persist.tile([P, 1], f32r, tag="ones")
            qT_sb = [persist.tile([P, HPC, S], f32r, tag=f"qT{b}", name=f"qT{b}") for b in range(B)]
            kT_sb = [persist.tile([P, S], f32r, tag=f"kT{b}", name=f"kT{b}") for b in range(B)]
            v_sb = [persist.tile([P, S], f32r, tag=f"v{b}", name=f"v{b}") for b in range(B)]

            nc.sync.dma_start(out=wq_sb[:], in_=wq.rearrange("(c p) f -> p c f", p=P))
            nc.sync.dma_start(out=wk_sb[:], in_=wk.rearrange("(c p) f -> p c f", p=P))
            nc.sync.dma_start(out=wv_sb[:], in_=wv.rearrange("(c p) f -> p c f", p=P))
            nc.gpsimd.dma_start(out=ident[:], in_=ident_d[:])
            nc.gpsimd.dma_start(out=ident32[:], in_=ident32_d[:])
            nc.gpsimd.dma_start(out=ones_sb[:], in_=ones_d[:])

            # ---------------- phase 1: Q/K/V projections ----------------
            with tc.tile_pool(name="xt", bufs=2) as xpool, \
                 tc.tile_pool(name="vst", bufs=2) as vstage, \
                 tc.tile_pool(name="ppsum", bufs=4, space="PSUM") as ppsum, \
                 tc.tile_pool(name="tpsum", bufs=2, space="PSUM") as tpsum:
                XSUB = 4                      # hid chunks per xt sub-tile
                NSUB = NCH // XSUB
                for b in range(B):
                    for t0 in range(0, S, TT):
                        xts = []
                        for s in range(NSUB):
                            xs = xpool.tile([P, XSUB, TT], f32r, tag=f"xt{s}",
                                            name=f"xt{s}_{b}_{t0}")
                            eng = nc.gpsimd if s % 2 == 0 else nc.sync
                            eng.dma_start(
                                out=xs[:],
                                in_=xT[b, s * XSUB * P:(s + 1) * XSUB * P, t0:t0 + TT]
                                .rearrange("(c p) t -> p c t", p=P),
                            )
                            xts.append(xs)
                        xt = None
                        for h in range(HPC):
                            ps = ppsum.tile([P, TT], f32, tag="pp")
                            for c in range(NCH):
                                nc.tensor.matmul(
                                    ps[:],
                                    lhsT=wq_sb[:, c, h * HD:(h + 1) * HD],
                                    rhs=xts[c // XSUB][:, c % XSUB, :],
                                    start=(c == 0), stop=(c == NCH - 1),
                                )
                            nc.scalar.mul(out=qT_sb[b][:, h, t0:t0 + TT], in_=ps[:], mul=1.0)
                        ps = ppsum.tile([P, TT], f32, tag="pp")
                        for c in range(NCH):
                            nc.tensor.matmul(
                                ps[:], lhsT=wk_sb[:, c, :], rhs=xts[c // XSUB][:, c % XSUB, :],
                                start=(c == 0), stop=(c == NCH - 1),
                            )
                        nc.scalar.mul(out=kT_sb[b][:, t0:t0 + TT], in_=ps[:], mul=1.0)
                        ps = ppsum.tile([P, TT], f32, tag="pp")
                        for c in range(NCH):
                            nc.tensor.matmul(
                                ps[:], lhsT=wv_sb[:, c, :], rhs=xts[c // XSUB][:, c % XSUB, :],
                                start=(c == 0), stop=(c == NCH - 1),
                            )
                        vt = vstage.tile([P, TT], f32r, tag="vt")
                        nc.vector.tensor_copy(vt[:], ps[:])
                        for j in range(TT // P):
                            tp = tpsum.tile([P, P], f32r, tag="tp")
                            nc.tensor.transpose(tp[:], vt[:, j * P:(j + 1) * P], ident[:])
                            kc = t0 // P + j
                            nc.vector.tensor_copy(v_sb[b][:, kc * HD: (kc + 1) * HD], tp[:])

            # ---------------- phase 2+3: attention ----------------
            with tc.tile_pool(name="mask", bufs=4) as mpool, \
                 tc.tile_pool(name="et", bufs=1) as epool, \
                 tc.tile_pool(name="ot", bufs=2) as otpool, \
                 tc.tile_pool(name="small", bufs=8) as small, \
                 tc.tile_pool(name="spsum", bufs=3, space="PSUM") as spsum, \
                 tc.tile_pool(name="opsum", bufs=2, space="PSUM") as opsum, \
                 tc.tile_pool(name="supsum", bufs=1, space="PSUM") as supsum, \
                 tc.tile_pool(name="tpsum2", bufs=1, space="PSUM") as tpsum2:
                for b in range(B):
                    for q0 in range(0, S, QT):
                        act = active_kchunks(q0)
                        msk = masked_kchunks(q0)
                        et = [epool.tile([P, NKC, QT], f32r, tag=f"et{h}", name=f"et{h}_{b}_{q0}")
                              for h in range(HPC)]
                        # scores + exp, per k-chunk (mask tile shared by both heads)
                        for kc in act:
                            if kc in msk:
                                mt = mpool.tile([P, QT], bf16, tag="mt")
                                nc.sync.dma_start(
                                    out=mt[:], in_=maskT[b, kc * P:(kc + 1) * P, q0:q0 + QT]
                                )
                            for h in range(HPC):
                                sp = spsum.tile([P, QT], f32, tag="sp")
                                nc.tensor.matmul(
                                    sp[:],
                                    lhsT=kT_sb[b][:, kc * P:(kc + 1) * P],
                                    rhs=qT_sb[b][:, h, q0:q0 + QT],
                                    start=True, stop=True,
                                )
                                if kc in msk:
                                    nc.vector.tensor_add(out=sp[:], in0=sp[:], in1=mt[:])
                                nc.scalar.activation(out=et[h][:, kc, :], in_=sp[:], func=Exp)
                        # attn @ V (out^T form)
                        po = {}
                        for h in range(HPC):
                            po[h] = opsum.tile([P, QT], f32, tag="po", name=f"po{h}_{b}_{q0}")
                            for kc in act:
                                nc.tensor.matmul(
                                    po[h][:],
                                    lhsT=v_sb[b][:, kc * HD:(kc + 1) * HD],
                                    rhs=et[h][:, kc, :],
                                    start=(kc == act[0]), stop=(kc == act[-1]),
                                )
                        srow2 = small.tile([32 * (HPC - 1) + 1, QT], f32, tag="srow2")
                        oT = {}
                        for h in range(HPC):
                            psums = supsum.tile([P, QT], f32, tag="ps_sums")
                            for kc in act:
                                nc.tensor.matmul(
                                    psums[:1, :],
                                    lhsT=ones_sb[:, :1],
                                    rhs=et[h][:, kc, :],
                                    start=(kc == act[0]), stop=(kc == act[-1]),
                                )
                            oT[h] = otpool.tile([P, QT], f32, tag=f"oT{h}", name=f"oT{h}_{b}_{q0}")
                            nc.scalar.mul(out=oT[h][:], in_=po[h][:], mul=1.0)
                            nc.scalar.mul(out=srow2[32 * h:32 * h + 1, :], in_=psums[:1, :], mul=1.0)
                        # flip [d, q] -> [q, d] and normalize per-q
                        for qj in range(QT // P):
                            tps = tpsum2.tile([P, P], f32, tag="tps")
                            nw = 32 * (HPC - 1) + 1
                            nc.tensor.transpose(
                                tps[:, :nw], srow2[:, qj * P:(qj + 1) * P],
                                ident32[:nw, :nw],
                            )
                            rc = small.tile([P, HPC], f32, tag="rc")
                            for h in range(HPC):
                                nc.vector.reciprocal(rc[:, h:h + 1], tps[:, 32 * h:32 * h + 1])
                            for h in range(HPC):
                                tpo = tpsum2.tile([P, P], f32, tag="tpo")
                                nc.tensor.transpose(
                                    tpo[:], oT[h][:, qj * P:(qj + 1) * P], ident32[:]
                                )
                                ob = small.tile([P, HD], f32, tag="ob")
                                nc.vector.tensor_scalar_mul(ob[:], tpo[:], rc[:, h:h + 1])
                                nc.sync.dma_start(
                                    out=out[b, q0 + qj * P: q0 + (qj + 1) * P, h * HD:(h + 1) * HD],
                                    in_=ob[:],
                                )

    nc.compile()
    return nc


def get_nc(variant="general"):
    if variant not in _CACHE:
        _CACHE[variant] = _build_nc(variant)
    return _CACHE[variant]


def detect_variant(attention_mask):
    m = np.asarray(attention_mask, dtype=np.float32)[:, 0]   # [B, S, S] (q, k)
    if not np.any(m):
        return "zeros"
    # causal: zero on/below the diagonal, <= -1e8 strictly above
    kk = np.arange(S)
    lower = kk[None, :] <= kk[:, None]                       # [S(q), S(k)]
    for b in range(m.shape[0]):
        if np.any(m[b][lower] != 0.0):
            return "general"
        if np.any(m[b][~lower] > -1e8):
            return "general"
    return "causal"


def make_in_maps(hidden_states, attention_mask, Wq, Wk, Wv):
    import ml_dtypes

    xT = np.ascontiguousarray(
        np.asarray(hidden_states, dtype=np.float32).transpose(0, 2, 1)
    )
    mT = np.ascontiguousarray(
        np.asarray(attention_mask, dtype=np.float32)[:, 0].transpose(0, 2, 1)
    ).astype(ml_dtypes.bfloat16)
    wq_s = (np.asarray(Wq, dtype=np.float32) / math.sqrt(HD)).astype(np.float32)
    wk = np.asarray(Wk, dtype=np.float32)
    wv = np.asarray(Wv, dtype=np.float32)
    ident = np.eye(P, dtype=np.float32)
    ones = np.ones((P, 1), dtype=np.float32)

    in_maps = []
    for c in range(NCORES):
        kv = c // 2
        in_maps.append({
            "xT": xT,
            "wq": np.ascontiguousarray(wq_s[:, c * FPC:(c + 1) * FPC]),
            "wk": np.ascontiguousarray(wk[:, kv * KVW:(kv + 1) * KVW]),
            "wv": np.ascontiguousarray(wv[:, kv * KVW:(kv + 1) * KVW]),
            "maskT": mT,
            "ident": ident,
            "ident32": ident,
            "ones": ones,
        })
    return in_maps


def kernel(hidden_states, attention_mask, Wq, Wk, Wv):
    from concourse.bass_utils import run_bass_kernel_spmd

    variant = detect_variant(attention_mask)
    nc = get_nc(variant)
    in_maps = make_in_maps(hidden_states, attention_mask, Wq, Wk, Wv)
    res = run_bass_kernel_spmd(nc, in_maps, core_ids=list(range(NCORES)))
    outs = [res.results[c]["out"] for c in range(NCORES)]
    return np.concatenate(outs, axis=2).astype(np.float32)


# revision 15
# speedup vs baseline: 1.2028x; 1.2028x over previous
"""Trainium2 Bass kernel for CheemsNonWoAttention (GQA attention, no output proj).

Sharding: 16 q-heads across 8 cores (2 q-heads + their shared kv-head per
core), SPMD with no collectives.  Each core computes its slice of the output
hidden dim; the host concatenates.

Math notes:
  - The reference's logn scale is max(log(65..80)/log(256), 1) == 1.0 -> no-op.
  - 1/sqrt(HD) score scale is folded into Wq on the host.
  - Softmax runs without max-subtraction (scores ~ N(0,1) + additive mask;
    exp underflows to 0 for very negative masks, which is exactly right).
    Scores are computed transposed, sT[k, q], so exp(sT) feeds attn@V
    directly as the moving operand (V chunks stationary), denominators come
    from a ones-vector matmul, and only the final [d, q] -> [q, d] flip
    needs PE transposes.
  - Matmuls run in float32r (TF32-like, ~1.5e-4 rms rel err).
  - The host inspects the mask and dispatches one of three compiled
    variants: "causal" (skips fully-masked k-chunks and the mask add on
    fully-unmasked ones), "zeros" (no mask work at all), "general"
    (arbitrary additive mask).
"""

import sys

if "/opt/trn_rl_repo" not in sys.path:
    sys.path.insert(0, "/opt/trn_rl_repo")

import math
import numpy as np

B, S, HID = 2, 2048, 2048
NH, NKV, HD = 16, 4, 128
NCORES = 8
HPC = NH // NCORES          # q heads per core
FPC = HPC * HD              # output features per core
KVW = HD                    # kv head width per core
P = 128
NCH = HID // P              # hid chunks (contraction tiles)
TT = 512                    # token tile, phase 1
QT = 512                    # q tile, phase 2
NKC = S // P                # k chunks

_CACHE = {}


def _patch_ldw_opt():
    # walrus's LDWEIGHTS dedup/overlap pass is off by default in the driver
    # args; it is worth ~13% end-to-end here (weight loads dominate fp32r
    # matmul issue otherwise).  Results verified identical with it on.
    import concourse.bass_utils as bu

    if getattr(bu, "_ldw_opt_patched", False):
        return
    orig = bu.run_command

    def patched(argv, **kw):
        argv = ["--enable-ldw-opt=true" if a == "--enable-ldw-opt=false" else a
                for a in argv]
        return orig(argv, **kw)

    bu.run_command = patched
    bu._ldw_opt_patched = True


def _build_nc(variant):
    _patch_ldw_opt()
    import concourse.bacc as bacc
    from concourse import mybir
    from concourse.tile import TileContext

    f32 = mybir.dt.float32
    f32r = mybir.dt.float32r
    bf16 = mybir.dt.bfloat16
    Exp = mybir.ActivationFunctionType.Exp

    nc = bacc.Bacc("TRN2", target_bir_lowering=False, debug=False, num_devices=NCORES)
    xT = nc.dram_tensor("xT", [B, HID, S], f32r, kind="ExternalInput").ap()
    wq = nc.dram_tensor("wq", [HID, FPC], f32r, kind="ExternalInput").ap()
    wk = nc.dram_tensor("wk", [HID, KVW], f32r, kind="ExternalInput").ap()
    wv = nc.dram_tensor("wv", [HID, KVW], f32r, kind="ExternalInput").ap()
    maskT = nc.dram_tensor("maskT", [B, S, S], bf16, kind="ExternalInput").ap()
    ident_d = nc.dram_tensor("ident", [P, P], f32r, kind="ExternalInput").ap()
    ident32_d = nc.dram_tensor("ident32", [P, P], f32, kind="ExternalInput").ap()
    ones_d = nc.dram_tensor("ones", [P, 1], f32r, kind="ExternalInput").ap()
    out = nc.dram_tensor("out", [B, S, FPC], f32, kind="ExternalOutput").ap()

    def active_kchunks(q0):
        if variant == "causal":
            return list(range(q0 // P + QT // P))
        return list(range(NKC))

    def masked_kchunks(q0):
        if variant == "causal":
            return set(range(q0 // P, q0 // P + QT // P))
        if variant == "zeros":
            return set()
        return set(range(NKC))

    with TileContext(nc) as tc:
        with tc.tile_pool(name="persist", bufs=1) as persist:
            wq_sb = persist.tile([P, NCH, FPC], f32r, tag="wq")
            wk_sb = persist.tile([P, NCH, KVW], f32r, tag="wk")
            wv_sb = persist.tile([P, NCH, KVW], f32r, tag="wv")
            ident = persist.tile([P, P], f32r, tag="ident")
            ident32 = persist.tile([P, P], f32, tag="ident32")
            ones_sb = persist.tile([P, 1], f32r, tag="ones")
            qT_sb = [persist.tile([P, HPC, S], f32r, tag=f"qT{b}", name=f"qT{b}") for b in range(B)]
            kT_sb = [persist.tile([P, S], f32r, tag=f"kT{b}", name=f"kT{b}") for b in range(B)]
            v_sb = [persist.tile([P, S], f32r, tag=f"v{b}", name=f"v{b}") for b in range(B)]

            nc.sync.dma_start(out=wq_sb[:], in_=wq.rearrange("(c p) f -> p c f", p=P))
            nc.sync.dma_start(out=wk_sb[:], in_=wk.rearrange("(c p) f -> p c f", p=P))
            nc.sync.dma_start(out=wv_sb[:], in_=wv.rearrange("(c p) f -> p c f", p=P))
            nc.gpsimd.dma_start(out=ident[:], in_=ident_d[:])
            nc.gpsimd.dma_start(out=ident32[:], in_=ident32_d[:])
            nc.gpsimd.dma_start(out=ones_sb[:], in_=ones_d[:])

            # ---------------- phase 1: Q/K/V projections ----------------
            with tc.tile_pool(name="xt", bufs=2) as xpool, \
                 tc.tile_pool(name="vst", bufs=2) as vstage, \
                 tc.tile_pool(name="ppsum", bufs=4, space="PSUM") as ppsum, \
                 tc.tile_pool(name="tpsum", bufs=2, space="PSUM") as tpsum:
                XSUB = 4                      # hid chunks per xt sub-tile
                NSUB = NCH // XSUB
                for b in range(B):
                    for t0 in range(0, S, TT):
                        xts = []
                        for s in range(NSUB):
                            xs = xpool.tile([P, XSUB, TT], f32r, tag=f"xt{s}",
                                            name=f"xt{s}_{b}_{t0}")
                            nc.sync.dma_start(
                                out=xs[:],
                                in_=xT[b, s * XSUB * P:(s + 1) * XSUB * P, t0:t0 + TT]
                                .rearrange("(c p) t -> p c t", p=P),
                            )
                            xts.append(xs)
                        xt = None
                        for h in range(HPC):
                            ps = ppsum.tile([P, TT], f32, tag="pp")
                            for c in range(NCH):
                                nc.tensor.matmul(
                                    ps[:],
                                    lhsT=wq_sb[:, c, h * HD:(h + 1) * HD],
                                    rhs=xts[c // XSUB][:, c % XSUB, :],
                                    start=(c == 0), stop=(c == NCH - 1),
                                )
                            nc.scalar.mul(out=qT_sb[b][:, h, t0:t0 + TT], in_=ps[:], mul=1.0)
                        ps = ppsum.tile([P, TT], f32, tag="pp")
                        for c in range(NCH):
                            nc.tensor.matmul(
                                ps[:], lhsT=wk_sb[:, c, :], rhs=xts[c // XSUB][:, c % XSUB, :],
                                start=(c == 0), stop=(c == NCH - 1),
                            )
                        nc.scalar.mul(out=kT_sb[b][:, t0:t0 + TT], in_=ps[:], mul=1.0)
                        ps = ppsum.tile([P, TT], f32, tag="pp")
                        for c in range(NCH):
                            nc.tensor.matmul(
                                ps[:], lhsT=wv_sb[:, c, :], rhs=xts[c // XSUB][:, c % XSUB, :],
                                start=(c == 0), stop=(c == NCH - 1),
                            )
                        vt = vstage.tile([P, TT], f32r, tag="vt")
                        nc.vector.tensor_copy(vt[:], ps[:])
                        for j in range(TT // P):
                            tp = tpsum.tile([P, P], f32r, tag="tp")
                            nc.tensor.transpose(tp[:], vt[:, j * P:(j + 1) * P], ident[:])
                            kc = t0 // P + j
                            nc.vector.tensor_copy(v_sb[b][:, kc * HD: (kc + 1) * HD], tp[:])

            # ---------------- phase 2+3: attention ----------------
            with tc.tile_pool(name="mask", bufs=4) as mpool, \
                 tc.tile_pool(name="et", bufs=1) as epool, \
                 tc.tile_pool(name="ot", bufs=2) as otpool, \
                 tc.tile_pool(name="small", bufs=8) as small, \
                 tc.tile_pool(name="spsum", bufs=3, space="PSUM") as spsum, \
                 tc.tile_pool(name="opsum", bufs=2, space="PSUM") as opsum, \
                 tc.tile_pool(name="supsum", bufs=1, space="PSUM") as supsum, \
                 tc.tile_pool(name="tpsum2", bufs=1, space="PSUM") as tpsum2:
                for b in range(B):
                    for q0 in range(0, S, QT):
                        act = active_kchunks(q0)
                        msk = masked_kchunks(q0)
                        et = [epool.tile([P, NKC, QT], f32r, tag=f"et{h}", name=f"et{h}_{b}_{q0}")
                              for h in range(HPC)]
                        # scores + exp, per k-chunk (mask tile shared by both heads)
                        for kc in act:
                            if kc in msk:
                                mt = mpool.tile([P, QT], bf16, tag="mt")
                                nc.sync.dma_start(
                                    out=mt[:], in_=maskT[b, kc * P:(kc + 1) * P, q0:q0 + QT]
                                )
                            for h in range(HPC):
                                sp = spsum.tile([P, QT], f32, tag="sp")
                                nc.tensor.matmul(
                                    sp[:],
                                    lhsT=kT_sb[b][:, kc * P:(kc + 1) * P],
                                    rhs=qT_sb[b][:, h, q0:q0 + QT],
                                    start=True, stop=True,
                                )
                                if kc in msk:
                                    nc.vector.tensor_add(out=sp[:], in0=sp[:], in1=mt[:])
                                nc.scalar.activation(out=et[h][:, kc, :], in_=sp[:], func=Exp)
                        # attn @ V (out^T form)
                        po = {}
                        for h in range(HPC):
                            po[h] = opsum.tile([P, QT], f32, tag="po", name=f"po{h}_{b}_{q0}")
                            for kc in act:
                                nc.tensor.matmul(
                                    po[h][:],
                                    lhsT=v_sb[b][:, kc * HD:(kc + 1) * HD],
                                    rhs=et[h][:, kc, :],
                                    start=(kc == act[0]), stop=(kc == act[-1]),
                                )
                        srow2 = small.tile([32 * (HPC - 1) + 1, QT], f32, tag="srow2")
                        oT = {}
                        for h in range(HPC):
                            psums = supsum.tile([P, QT], f32, tag="ps_sums")
                            for kc in act:
                                nc.tensor.matmul(
                                    psums[:1, :],
                                    lhsT=ones_sb[:, :1],
                                    rhs=et[h][:, kc, :],
                                    start=(kc == act[0]), stop=(kc == act[-1]),
                                )
                            oT[h] = otpool.tile([P, QT], f32, tag=f"oT{h}", name=f"oT{h}_{b}_{q0}")
                            nc.scalar.mul(out=oT[h][:], in_=po[h][:], mul=1.0)
                            nc.scalar.mul(out=srow2[32 * h:32 * h + 1, :], in_=psums[:1, :], mul=1.0)
                        # flip [d, q] -> [q, d] and normalize per-q
                        for qj in range(QT // P):
                            tps = tpsum2.tile([P, P], f32, tag="tps")
                            nw = 32 * (HPC - 1) + 1
                            nc.tensor.transpose(
                                tps[:, :nw], srow2[:, qj * P:(qj + 1) * P],
                                ident32[:nw, :nw],
                            )
                            rc = small.tile([P, HPC], f32, tag="rc")
                            for h in range(HPC):
                                nc.vector.reciprocal(rc[:, h:h + 1], tps[:, 32 * h:32 * h + 1])
                            for h in range(HPC):
                                tpo = tpsum2.tile([P, P], f32, tag="tpo")
                                nc.tensor.transpose(
                                    tpo[:], oT[h][:, qj * P:(qj + 1) * P], ident32[:]
                                )
                                ob = small.tile([P, HD], f32, tag="ob")
                                nc.vector.tensor_scalar_mul(ob[:], tpo[:], rc[:, h:h + 1])
                                nc.sync.dma_start(
                                    out=out[b, q0 + qj * P: q0 + (qj + 1) * P, h * HD:(h + 1) * HD],
                                    in_=ob[:],
                                )

    nc.compile()
    return nc


def get_nc(variant="general"):
    if variant not in _CACHE:
        _CACHE[variant] = _build_nc(variant)
    return _CACHE[variant]


def detect_variant(attention_mask):
    m = np.asarray(attention_mask, dtype=np.float32)[:, 0]   # [B, S, S] (q, k)
    if not np.any(m):
        return "zeros"
    # causal: zero on/below the diagonal, <= -1e8 strictly above
    kk = np.arange(S)
    lower = kk[None, :] <= kk[:, None]                       # [S(q), S(k)]
    for b in range(m.shape[0]):
        if np.any(m[b][lower] != 0.0):
            return "general"
        if np.any(m[b][~lower] > -1e8):
            return "general"
    return "causal"


def make_in_maps(hidden_states, attention_mask, Wq, Wk, Wv):
    import ml_dtypes

    xT = np.ascontiguousarray(
        np.asarray(hidden_states, dtype=np.float32).transpose(0, 2, 1)
    )
    mT = np.ascontiguousarray(
        np.asarray(attention_mask, dtype=np.float32)[:, 0].transpose(0, 2, 1)
    ).astype(ml_dtypes.bfloat16)
    wq_s = (np.asarray(Wq, dtype=np.float32) / math.sqrt(HD)).astype(np.float32)
    wk = np.asarray(Wk, dtype=np.float32)
    wv = np.asarray(Wv, dtype=np.float32)
    ident = np.eye(P, dtype=np.float32)
    ones = np.ones((P, 1), dtype=np.float32)

    in_maps = []
    for c in range(NCORES):
        kv = c // 2
        in_maps.append({
            "xT": xT,
            "wq": np.ascontiguousarray(wq_s[:, c * FPC:(c + 1) * FPC]),
            "wk": np.ascontiguousarray(wk[:, kv * KVW:(kv + 1) * KVW]),
            "wv": np.ascontiguousarray(wv[:, kv * KVW:(kv + 1) * KVW]),
            "maskT": mT,
            "ident": ident,
            "ident32": ident,
            "ones": ones,
        })
    return in_maps


def kernel(hidden_states, attention_mask, Wq, Wk, Wv):
    from concourse.bass_utils import run_bass_kernel_spmd

    variant = detect_variant(attention_mask)
    nc = get_nc(variant)
    in_maps = make_in_maps(hidden_states, attention_mask, Wq, Wk, Wv)
    res = run_bass_kernel_spmd(nc, in_maps, core_ids=list(range(NCORES)))
    outs = [res.results[c]["out"] for c in range(NCORES)]
    return np.concatenate(outs, axis=2).astype(np.float32)
